# revision 1
# baseline (speedup 1.0000x reference)
"""MoE (8 experts, top-2) TRN2 kernel — expert-parallel, dense-masked variant.

Core i holds expert i's weights (bf16); x replicated (fp32 transposed for the
fp32 gating matmul + bf16 transposed for the FFN). Each core computes fp32
gating for all tokens, derives its expert's top-2-masked softmax weight
comb_e[t], runs the FFN on ALL tokens in bf16, scales rows by comb_e and
writes a partial output. Host sums the 8 partials.

Gating columns are permuted per core so "my expert" is always column 0.
"""

import sys
import types

sys.path.insert(0, "/opt/trn_rl_repo")

import numpy as np
import ml_dtypes

try:
    import antenv.axon_hooks  # noqa: F401
except ImportError:
    try:
        import antenv
        import trn_agent_boot.trn_boot as _tb

        _hook = _tb._ntff_profile_via_ctypes("/opt/axon/libaxon_pjrt.so")
        _m = types.ModuleType("antenv.axon_hooks")
        _m.get_axon_ntff_profile_hook = lambda: _hook
        _m.set_axon_ntff_profile_hook = lambda h: None
        sys.modules["antenv.axon_hooks"] = _m
        antenv.axon_hooks = _m
    except Exception:
        pass

import concourse.bacc as bacc
import concourse.mybir as mybir
from concourse import bass, bass_utils
from concourse.tile import TileContext
from concourse.masks import make_identity

E = 8
H = 512
F = 2048
T = 8 * 2048
BF16 = mybir.dt.bfloat16
F32 = mybir.dt.float32

_CACHE = {}
LAST_RESULT = None


def _build():
    nc = bacc.Bacc(debug=False)

    xt = nc.declare_dram_parameter("xt", [128, 4, T], F32, isOutput=False)
    xbt = nc.declare_dram_parameter("xbt", [128, 4, T], BF16, isOutput=False)
    wg = nc.declare_dram_parameter("wg", [128, 4, E], F32, isOutput=False)
    bg = nc.declare_dram_parameter("bg", [E, 1], F32, isOutput=False)
    w1 = nc.declare_dram_parameter("w1", [128, 4, F], BF16, isOutput=False)
    b1t = nc.declare_dram_parameter("b1t", [128, F // 128], F32, isOutput=False)
    w2 = nc.declare_dram_parameter("w2", [128, F // 128, H], BF16, isOutput=False)
    b2r = nc.declare_dram_parameter("b2r", [128, H], F32, isOutput=False)
    ypart = nc.declare_dram_parameter("ypart", [T, H], F32, isOutput=True)

    with TileContext(nc) as tc:
        with (
            tc.tile_pool(name="const", bufs=1) as constp,
            tc.tile_pool(name="work", bufs=4) as work,
            tc.tile_pool(name="gate", bufs=3) as gate,
            tc.tile_pool(name="big", bufs=1) as bigp,
            tc.tile_pool(name="psA", bufs=3, space="PSUM") as psA,
            tc.tile_pool(name="psB", bufs=3, space="PSUM") as psB,
            tc.tile_pool(name="psT", bufs=2, space="PSUM") as psT,
        ):
            ident = constp.tile([128, 128], F32)
            make_identity(nc, ident[:])
            wg_sb = constp.tile([128, 4, E], F32)
            nc.sync.dma_start(out=wg_sb[:], in_=wg[:])
            bg_sb = constp.tile([E, 1], F32)
            nc.sync.dma_start(out=bg_sb[:], in_=bg[:])
            w1_sb = constp.tile([128, 4, F], BF16)
            nc.sync.dma_start(out=w1_sb[:], in_=w1[:])
            b1_sb = constp.tile([128, F // 128], F32)
            nc.sync.dma_start(out=b1_sb[:], in_=b1t[:])
            w2_sb = constp.tile([128, F // 128, H], BF16)
            nc.sync.dma_start(out=w2_sb[:], in_=w2[:])
            b2_sb = constp.tile([128, H], F32)
            nc.sync.dma_start(out=b2_sb[:], in_=b2r[:])

            comb_all = bigp.tile([128, 128], F32)  # [token%128, token//128]

            # ---- gating (fp32) + top-2 routing for one 2048-token group
            def emit_gate(og):
                lsbs = []
                for sg in range(4):
                    g = og * 4 + sg
                    xt_sb = gate.tile([128, 4, 512], F32, tag="xt")
                    for c in range(4):
                        nc.sync.dma_start(
                            out=xt_sb[:, c, :], in_=xt[:, c, g * 512 : (g + 1) * 512]
                        )
                    lp = psA.tile([E, 512], F32, tag="mmA")
                    for c in range(4):
                        nc.tensor.matmul(
                            lp[:],
                            wg_sb[:, c, :],
                            xt_sb[:, c, :],
                            start=(c == 0),
                            stop=(c == 3),
                        )
                    l_sb = gate.tile([E, 512], F32, tag="lsb")
                    nc.vector.tensor_scalar_add(l_sb[:], lp[:], bg_sb[:, 0:1])
                    lsbs.append(l_sb)
                lt = gate.tile([128, 16, E], F32, tag="lt")
                for k in range(16):
                    tp = psT.tile([128, E], F32, tag="tp")
                    nc.tensor.transpose(
                        tp[:],
                        lsbs[k // 4][:, (k % 4) * 128 : (k % 4 + 1) * 128],
                        ident[:E, :E],
                    )
                    nc.vector.tensor_copy(out=lt[:, k, :], in_=tp[:])
                m1 = gate.tile([128, 16], F32, tag="m1")
                nc.vector.tensor_reduce(
                    m1[:], lt[:], axis=mybir.AxisListType.X, op=mybir.AluOpType.max
                )
                lsh = gate.tile([128, 16, E], F32, tag="lsh")
                nc.vector.tensor_tensor(
                    out=lsh[:],
                    in0=lt[:],
                    in1=m1[:].to_broadcast([128, 16, E]),
                    op=mybir.AluOpType.subtract,
                )
                ex = gate.tile([128, 16, E], F32, tag="ex")
                nc.scalar.activation(ex[:], lsh[:], mybir.ActivationFunctionType.Exp)
                ssum = gate.tile([128, 16], F32, tag="ssum")
                nc.vector.tensor_reduce(
                    ssum[:], ex[:], axis=mybir.AxisListType.X, op=mybir.AluOpType.add
                )
                rcp = gate.tile([128, 16], F32, tag="rcp")
                nc.vector.reciprocal(rcp[:], ssum[:])
                eq = gate.tile([128, 16, E], F32, tag="eq")
                nc.vector.tensor_scalar(
                    eq[:], lsh[:], 0.0, None, op0=mybir.AluOpType.is_ge
                )
                msk = gate.tile([128, 16, E], F32, tag="msk")
                nc.vector.scalar_tensor_tensor(
                    out=msk[:],
                    in0=eq[:],
                    scalar=-1e30,
                    in1=lsh[:],
                    op0=mybir.AluOpType.mult,
                    op1=mybir.AluOpType.add,
                )
                t2 = gate.tile([128, 16], F32, tag="t2")
                nc.vector.tensor_reduce(
                    t2[:], msk[:], axis=mybir.AxisListType.X, op=mybir.AluOpType.max
                )
                sel = gate.tile([128, 16, E], F32, tag="sel")
                nc.vector.tensor_tensor(
                    out=sel[:],
                    in0=lsh[:],
                    in1=t2[:].to_broadcast([128, 16, E]),
                    op=mybir.AluOpType.is_ge,
                )
                pm = gate.tile([128, 16, E], F32, tag="pm")
                nc.vector.tensor_tensor(
                    out=pm[:], in0=ex[:], in1=sel[:], op=mybir.AluOpType.mult
                )
                cmb = gate.tile([128, 16, E], F32, tag="cmb")
                nc.vector.tensor_tensor(
                    out=cmb[:],
                    in0=pm[:],
                    in1=rcp[:].to_broadcast([128, 16, E]),
                    op=mybir.AluOpType.mult,
                )
                nc.vector.tensor_copy(
                    out=comb_all[:, og * 16 : (og + 1) * 16], in_=cmb[:, :, 0]
                )

            # ---- FFN (bf16) for one 512-token group
            def emit_ffn(g):
                xg_sb = work.tile([128, 4, 512], BF16, tag="xg")
                for c in range(4):
                    nc.sync.dma_start(
                        out=xg_sb[:, c, :], in_=xbt[:, c, g * 512 : (g + 1) * 512]
                    )
                hb = work.tile([128, F // 128, 512], BF16, tag="hb")
                for ft in range(F // 128):
                    hp = psA.tile([128, 512], F32, tag="mmA")
                    for hc in range(4):
                        nc.tensor.matmul(
                            hp[:],
                            w1_sb[:, hc, ft * 128 : (ft + 1) * 128],
                            xg_sb[:, hc, :],
                            start=(hc == 0),
                            stop=(hc == 3),
                        )
                    nc.scalar.activation(
                        hb[:, ft, :],
                        hp[:],
                        mybir.ActivationFunctionType.Gelu_apprx_tanh,
                        bias=b1_sb[:, ft : ft + 1],
                        scale=1.0,
                    )
                # second matmul emitted already token-major: lhsT = hT tile,
                # moving = W2 rows -> no output transposes needed
                for st in range(4):
                    yp = psB.tile([128, 512], F32, tag="mmB")
                    for fc in range(F // 128):
                        nc.tensor.matmul(
                            yp[:],
                            hb[:, fc, st * 128 : (st + 1) * 128],
                            w2_sb[:, fc, :],
                            start=(fc == 0),
                            stop=(fc == F // 128 - 1),
                        )
                    y_sb = work.tile([128, H], F32, tag="ysb")
                    nc.vector.tensor_tensor(
                        out=y_sb[:], in0=yp[:], in1=b2_sb[:], op=mybir.AluOpType.add
                    )
                    nc.vector.tensor_scalar_mul(
                        y_sb[:], y_sb[:], comb_all[:, 4 * g + st : 4 * g + st + 1]
                    )
                    nc.sync.dma_start(
                        out=ypart[g * 512 + st * 128 : g * 512 + (st + 1) * 128, :],
                        in_=y_sb[:],
                    )

            # interleave: gating block og feeds FFN groups 4*og..4*og+3; the
            # next gating block's xt DMAs hide under the previous FFN block.
            for og in range(T // 2048):
                emit_gate(og)
                for g in range(4 * og, 4 * og + 4):
                    emit_ffn(g)
    nc.compile()
    return nc


def _prep_inputs(x, Wg, bg, W1, b1, W2, b2):
    xf = np.ascontiguousarray(np.asarray(x, dtype=np.float32).reshape(T, H))
    Wg = np.asarray(Wg, dtype=np.float32)
    bg = np.asarray(bg, dtype=np.float32)
    W1 = np.asarray(W1, dtype=np.float32)
    b1 = np.asarray(b1, dtype=np.float32)
    W2 = np.asarray(W2, dtype=np.float32)
    b2 = np.asarray(b2, dtype=np.float32)

    xtq = np.ascontiguousarray(np.transpose(xf.T.reshape(4, 128, T), (1, 0, 2)))
    xbt = np.ascontiguousarray(xtq.astype(ml_dtypes.bfloat16))

    in_maps = []
    for e in range(E):
        perm = [e] + [j for j in range(E) if j != e]
        wg_p = Wg[:, perm]
        bg_p = bg[perm]
        in_maps.append(
            {
                "xt": xtq,
                "xbt": xbt,
                "wg": np.ascontiguousarray(
                    np.transpose(wg_p.reshape(4, 128, E), (1, 0, 2))
                ),
                "bg": np.ascontiguousarray(bg_p.reshape(E, 1)),
                "w1": np.ascontiguousarray(
                    np.transpose(W1[e].reshape(4, 128, F), (1, 0, 2)).astype(
                        ml_dtypes.bfloat16
                    )
                ),
                "b1t": np.ascontiguousarray(b1[e].reshape(F // 128, 128).T),
                "w2": np.ascontiguousarray(
                    np.transpose(W2[e].reshape(F // 128, 128, H), (1, 0, 2)).astype(
                        ml_dtypes.bfloat16
                    )
                ),
                "b2r": np.ascontiguousarray(
                    np.broadcast_to(b2[e][None, :], (128, H)).copy()
                ),
            }
        )
    return in_maps


def kernel(x, Wg, bg, W1, b1, W2, b2):
    global LAST_RESULT
    if "nc" not in _CACHE:
        _CACHE["nc"] = _build()
    nc = _CACHE["nc"]
    in_maps = _prep_inputs(x, Wg, bg, W1, b1, W2, b2)
    import os

    trace = bool(os.environ.get("BASS_TRACE"))
    res = bass_utils.run_bass_kernel_spmd(
        nc, in_maps, core_ids=list(range(E)), trace=trace
    )
    LAST_RESULT = res
    out = res.results[0]["ypart"].astype(np.float64)
    for e in range(1, E):
        out += res.results[e]["ypart"].astype(np.float64)
    return out.astype(np.float32).reshape(8, 2048, H)



# revision 2
# speedup vs baseline: 2.8594x; 2.8594x over previous
"""MoE (8 experts, top-2) TRN2 kernel — routed expert-parallel variant.

Sharding strategy (host = the shard/unshard glue): compute the top-2 routing
decision on host and shard tokens by expert id — core i receives exactly the
tokens routed to expert i (gathered, bf16, transposed), padded to a common
capacity C. Each core then computes, ON DEVICE, the gating softmax for its
tokens (to get the combine weight = raw softmax prob of its own expert), the
FFN in bf16, scales rows by the combine weight and writes y_part [C, H].
Host scatter-adds the per-expert partials back to token order (the unshard).

Per-core PE work is ~C/16384 of the dense-masked variant (C ≈ 4608 vs 16384).

Gating columns are permuted per core so "my expert" is always column 0.
"""

import sys
import types

sys.path.insert(0, "/opt/trn_rl_repo")

import numpy as np
import ml_dtypes

try:
    import antenv.axon_hooks  # noqa: F401
except ImportError:
    try:
        import antenv
        import trn_agent_boot.trn_boot as _tb

        _hook = _tb._ntff_profile_via_ctypes("/opt/axon/libaxon_pjrt.so")
        _m = types.ModuleType("antenv.axon_hooks")
        _m.get_axon_ntff_profile_hook = lambda: _hook
        _m.set_axon_ntff_profile_hook = lambda h: None
        sys.modules["antenv.axon_hooks"] = _m
        antenv.axon_hooks = _m
    except Exception:
        pass

import concourse.bacc as bacc
import concourse.mybir as mybir
from concourse import bass, bass_utils
from concourse.tile import TileContext
from concourse.masks import make_identity

E = 8
H = 512
F = 2048
T = 8 * 2048
BF16 = mybir.dt.bfloat16
F32 = mybir.dt.float32

_CACHE = {}
LAST_RESULT = None


def _build(C):
    """Bass program for one core: gating + FFN over C gathered tokens."""
    assert C % 512 == 0
    nc = bacc.Bacc(debug=False)

    xe = nc.declare_dram_parameter("xe", [128, 4, C], BF16, isOutput=False)
    wg = nc.declare_dram_parameter("wg", [128, 4, E], BF16, isOutput=False)
    bg = nc.declare_dram_parameter("bg", [E, 1], F32, isOutput=False)
    w1 = nc.declare_dram_parameter("w1", [128, 4, F], BF16, isOutput=False)
    b1t = nc.declare_dram_parameter("b1t", [128, F // 128], F32, isOutput=False)
    w2 = nc.declare_dram_parameter("w2", [128, F // 128, H], BF16, isOutput=False)
    b2r = nc.declare_dram_parameter("b2r", [128, H], F32, isOutput=False)
    ypart = nc.declare_dram_parameter("ypart", [C, H], F32, isOutput=True)

    with TileContext(nc) as tc:
        with (
            tc.tile_pool(name="const", bufs=1) as constp,
            tc.tile_pool(name="work", bufs=4) as work,
            tc.tile_pool(name="gate", bufs=3) as gate,
            tc.tile_pool(name="psA", bufs=3, space="PSUM") as psA,
            tc.tile_pool(name="psB", bufs=3, space="PSUM") as psB,
            tc.tile_pool(name="psT", bufs=2, space="PSUM") as psT,
        ):
            ident = constp.tile([128, 128], F32)
            make_identity(nc, ident[:])
            wg_sb = constp.tile([128, 4, E], BF16)
            nc.sync.dma_start(out=wg_sb[:], in_=wg[:])
            bg_sb = constp.tile([E, 1], F32)
            nc.sync.dma_start(out=bg_sb[:], in_=bg[:])
            # weights DMA'd in slices so the first FFN matmuls don't wait on
            # the whole tensor
            w1_sb = constp.tile([128, 4, F], BF16)
            for c in range(4):
                nc.sync.dma_start(out=w1_sb[:, c, :], in_=w1[:, c, :])
            b1_sb = constp.tile([128, F // 128], F32)
            nc.sync.dma_start(out=b1_sb[:], in_=b1t[:])
            w2_sb = constp.tile([128, F // 128, H], BF16)
            for fc in range(F // 128):
                nc.sync.dma_start(out=w2_sb[:, fc, :], in_=w2[:, fc, :])
            b2_sb = constp.tile([128, H], F32)
            nc.sync.dma_start(out=b2_sb[:], in_=b2r[:])

            def emit_chunk(g):
                # ---- load 512 gathered tokens (bf16, H on partitions)
                xg_sb = work.tile([128, 4, 512], BF16, tag="xg")
                for c in range(4):
                    nc.sync.dma_start(
                        out=xg_sb[:, c, :], in_=xe[:, c, g * 512 : (g + 1) * 512]
                    )

                # ---- gating: full softmax over 8 experts; comb = prob of
                # own expert (column 0) — membership already decided by host
                lp = psA.tile([E, 512], F32, tag="mmA")
                for c in range(4):
                    nc.tensor.matmul(
                        lp[:],
                        wg_sb[:, c, :],
                        xg_sb[:, c, :],
                        start=(c == 0),
                        stop=(c == 3),
                    )
                l_sb = gate.tile([E, 512], F32, tag="lsb")
                nc.vector.tensor_scalar_add(l_sb[:], lp[:], bg_sb[:, 0:1])
                lt = gate.tile([128, 4, E], F32, tag="lt")
                for k in range(4):
                    tp = psT.tile([128, E], F32, tag="tp")
                    nc.tensor.transpose(
                        tp[:],
                        l_sb[:, k * 128 : (k + 1) * 128],
                        ident[:E, :E],
                    )
                    nc.vector.tensor_copy(out=lt[:, k, :], in_=tp[:])
                m1 = gate.tile([128, 4], F32, tag="m1")
                nc.vector.tensor_reduce(
                    m1[:], lt[:], axis=mybir.AxisListType.X, op=mybir.AluOpType.max
                )
                lsh = gate.tile([128, 4, E], F32, tag="lsh")
                nc.vector.tensor_tensor(
                    out=lsh[:],
                    in0=lt[:],
                    in1=m1[:].to_broadcast([128, 4, E]),
                    op=mybir.AluOpType.subtract,
                )
                ex = gate.tile([128, 4, E], F32, tag="ex")
                nc.scalar.activation(ex[:], lsh[:], mybir.ActivationFunctionType.Exp)
                ssum = gate.tile([128, 4], F32, tag="ssum")
                nc.vector.tensor_reduce(
                    ssum[:], ex[:], axis=mybir.AxisListType.X, op=mybir.AluOpType.add
                )
                rcp = gate.tile([128, 4], F32, tag="rcp")
                nc.vector.reciprocal(rcp[:], ssum[:])
                comb = gate.tile([128, 4], F32, tag="cmb")
                nc.vector.tensor_tensor(
                    out=comb[:], in0=ex[:, :, 0], in1=rcp[:], op=mybir.AluOpType.mult
                )

                # ---- FFN (bf16)
                hb = work.tile([128, F // 128, 512], BF16, tag="hb")
                for ft in range(F // 128):
                    hp = psA.tile([128, 512], F32, tag="mmA")
                    for hc in range(4):
                        nc.tensor.matmul(
                            hp[:],
                            w1_sb[:, hc, ft * 128 : (ft + 1) * 128],
                            xg_sb[:, hc, :],
                            start=(hc == 0),
                            stop=(hc == 3),
                        )
                    nc.scalar.activation(
                        hb[:, ft, :],
                        hp[:],
                        mybir.ActivationFunctionType.Gelu_apprx_tanh,
                        bias=b1_sb[:, ft : ft + 1],
                        scale=1.0,
                    )
                # second matmul emitted token-major: lhsT = h tile,
                # moving = W2 rows -> no output transposes needed
                for st in range(4):
                    yp = psB.tile([128, 512], F32, tag="mmB")
                    for fc in range(F // 128):
                        nc.tensor.matmul(
                            yp[:],
                            hb[:, fc, st * 128 : (st + 1) * 128],
                            w2_sb[:, fc, :],
                            start=(fc == 0),
                            stop=(fc == F // 128 - 1),
                        )
                    y_sb = work.tile([128, H], F32, tag="ysb")
                    nc.vector.tensor_tensor(
                        out=y_sb[:], in0=yp[:], in1=b2_sb[:], op=mybir.AluOpType.add
                    )
                    nc.vector.tensor_scalar_mul(
                        y_sb[:], y_sb[:], comb[:, st : st + 1]
                    )
                    nc.sync.dma_start(
                        out=ypart[g * 512 + st * 128 : g * 512 + (st + 1) * 128, :],
                        in_=y_sb[:],
                    )

            for g in range(C // 512):
                emit_chunk(g)
    nc.compile()
    return nc


def _route(xf, Wg, bg):
    """Top-2 routing on host (fp32, same semantics as the reference)."""
    logits = xf @ Wg + bg
    m = logits.max(-1, keepdims=True)
    p = np.exp(logits - m)
    p /= p.sum(-1, keepdims=True)
    order = np.argsort(-p, axis=-1, kind="stable")
    topi = order[:, :2]
    mask = np.zeros_like(p, dtype=bool)
    np.put_along_axis(mask, topi, True, axis=-1)
    idx = [np.nonzero(mask[:, e])[0] for e in range(E)]
    return idx


def _prep_inputs(x, Wg, bg, W1, b1, W2, b2):
    xf = np.ascontiguousarray(np.asarray(x, dtype=np.float32).reshape(T, H))
    Wg = np.asarray(Wg, dtype=np.float32)
    bg = np.asarray(bg, dtype=np.float32)
    W1 = np.asarray(W1, dtype=np.float32)
    b1 = np.asarray(b1, dtype=np.float32)
    W2 = np.asarray(W2, dtype=np.float32)
    b2 = np.asarray(b2, dtype=np.float32)

    idx = _route(xf, Wg, bg)
    maxc = max(len(i) for i in idx)
    C = max(512, -(-maxc // 512) * 512)

    # x transposed to [128, 4, T] (h = c*128 + p), bf16
    xbt = np.ascontiguousarray(
        np.transpose(xf.T.reshape(4, 128, T), (1, 0, 2))
    ).astype(ml_dtypes.bfloat16)

    in_maps = []
    for e in range(E):
        xg = np.zeros((128, 4, C), dtype=ml_dtypes.bfloat16)
        xg[:, :, : len(idx[e])] = xbt[:, :, idx[e]]
        perm = [e] + [j for j in range(E) if j != e]
        wg_p = Wg[:, perm]
        bg_p = bg[perm]
        in_maps.append(
            {
                "xe": xg,
                "wg": np.ascontiguousarray(
                    np.transpose(wg_p.reshape(4, 128, E), (1, 0, 2))
                ).astype(ml_dtypes.bfloat16),
                "bg": np.ascontiguousarray(bg_p.reshape(E, 1)),
                "w1": np.ascontiguousarray(
                    np.transpose(W1[e].reshape(4, 128, F), (1, 0, 2)).astype(
                        ml_dtypes.bfloat16
                    )
                ),
                "b1t": np.ascontiguousarray(b1[e].reshape(F // 128, 128).T),
                "w2": np.ascontiguousarray(
                    np.transpose(W2[e].reshape(F // 128, 128, H), (1, 0, 2)).astype(
                        ml_dtypes.bfloat16
                    )
                ),
                "b2r": np.ascontiguousarray(
                    np.broadcast_to(b2[e][None, :], (128, H)).copy()
                ),
            }
        )
    return in_maps, idx, C


def kernel(x, Wg, bg, W1, b1, W2, b2):
    global LAST_RESULT
    in_maps, idx, C = _prep_inputs(x, Wg, bg, W1, b1, W2, b2)
    if C not in _CACHE:
        _CACHE[C] = _build(C)
    nc = _CACHE[C]
    import os

    trace = bool(os.environ.get("BASS_TRACE"))
    res = bass_utils.run_bass_kernel_spmd(
        nc, in_maps, core_ids=list(range(E)), trace=trace
    )
    LAST_RESULT = res
    out = np.zeros((T, H), dtype=np.float32)
    for e in range(E):
        out[idx[e]] += res.results[e]["ypart"][: len(idx[e])]
    return out.reshape(8, 2048, H)


# revision 3
# speedup vs baseline: 3.4465x; 1.2053x over previous
"""MoE (8 experts, top-2) TRN2 kernel — routed expert-parallel variant.

Sharding strategy (host = the shard/unshard glue): compute the top-2 routing
decision on host and shard tokens by expert id — core i receives exactly the
tokens routed to expert i (gathered, bf16, transposed), padded to a common
capacity C. Each core then computes, ON DEVICE, the gating softmax for its
tokens (to get the combine weight = raw softmax prob of its own expert), the
FFN in bf16, scales rows by the combine weight and writes y_part [C, H].
Host scatter-adds the per-expert partials back to token order (the unshard).

Structure: a gating prologue computes comb for all chunks first (single Exp
activation-table load, x parked in SBUF), then the FFN phase streams chunks
with a single Gelu table load and no input DMA. Weights ride the scalar-queue
DMAs (F-tile-major so the first FFN matmul only waits on one 128-col slice);
x and outputs ride the sync queue.

Gating columns are permuted per core so "my expert" is always column 0:
comb = softmax prob of col 0 = 1 / sum_j exp(l_j - l_0).
"""

import sys
import types

sys.path.insert(0, "/opt/trn_rl_repo")

import numpy as np
import ml_dtypes

try:
    import antenv.axon_hooks  # noqa: F401
except ImportError:
    try:
        import antenv
        import trn_agent_boot.trn_boot as _tb

        _hook = _tb._ntff_profile_via_ctypes("/opt/axon/libaxon_pjrt.so")
        _m = types.ModuleType("antenv.axon_hooks")
        _m.get_axon_ntff_profile_hook = lambda: _hook
        _m.set_axon_ntff_profile_hook = lambda h: None
        sys.modules["antenv.axon_hooks"] = _m
        antenv.axon_hooks = _m
    except Exception:
        pass

import concourse.bacc as bacc
import concourse.mybir as mybir
from concourse import bass, bass_utils
from concourse.tile import TileContext
from concourse.masks import make_identity

E = 8
H = 512
F = 2048
T = 8 * 2048
NFT = F // 128  # 16 F-tiles
BF16 = mybir.dt.bfloat16
F32 = mybir.dt.float32

_CACHE = {}
LAST_RESULT = None


def _build(C):
    """Bass program for one core: gating + FFN over C gathered tokens."""
    assert C % 512 == 0
    NG = C // 512
    nc = bacc.Bacc(debug=False)

    xe = nc.declare_dram_parameter("xe", [128, 4, C], BF16, isOutput=False)
    wg = nc.declare_dram_parameter("wg", [128, 4, E], BF16, isOutput=False)
    bg = nc.declare_dram_parameter("bg", [E, 1], F32, isOutput=False)
    # w1 F-tile-major: w1[p, ft, c, i] = W1[c*128+p, ft*128+i]
    w1 = nc.declare_dram_parameter("w1", [128, NFT, 4, 128], BF16, isOutput=False)
    b1t = nc.declare_dram_parameter("b1t", [128, NFT], F32, isOutput=False)
    w2 = nc.declare_dram_parameter("w2", [128, NFT, H], BF16, isOutput=False)
    b2r = nc.declare_dram_parameter("b2r", [128, H], F32, isOutput=False)
    ypart = nc.declare_dram_parameter("ypart", [C, H], F32, isOutput=True)

    with TileContext(nc) as tc:
        with (
            tc.tile_pool(name="const", bufs=1) as constp,
            tc.tile_pool(name="xres", bufs=1) as xres,
            tc.tile_pool(name="work", bufs=4) as work,
            tc.tile_pool(name="gate", bufs=3) as gate,
            tc.tile_pool(name="psA", bufs=3, space="PSUM") as psA,
            tc.tile_pool(name="psB", bufs=3, space="PSUM") as psB,
            tc.tile_pool(name="psT", bufs=2, space="PSUM") as psT,
        ):
            ident = constp.tile([128, 128], F32)
            make_identity(nc, ident[:])
            wg_sb = constp.tile([128, 4, E], BF16)
            nc.sync.dma_start(out=wg_sb[:], in_=wg[:])
            bg_sb = constp.tile([E, 1], F32)
            nc.sync.dma_start(out=bg_sb[:], in_=bg[:])
            # weights ride the scalar (Activation) DMA queue, sliced so the
            # first FFN matmuls only wait on their own slice
            b1_sb = constp.tile([128, NFT], F32)
            nc.scalar.dma_start(out=b1_sb[:], in_=b1t[:])
            b2_sb = constp.tile([128, H], F32)
            nc.scalar.dma_start(out=b2_sb[:], in_=b2r[:])
            w1_sb = constp.tile([128, NFT, 4, 128], BF16)
            for ft in range(NFT):
                nc.scalar.dma_start(out=w1_sb[:, ft, :, :], in_=w1[:, ft, :, :])
            w2_sb = constp.tile([128, NFT, H], BF16)
            for fc in range(NFT):
                nc.scalar.dma_start(out=w2_sb[:, fc, :], in_=w2[:, fc, :])

            # all x chunks parked in SBUF (sync queue)
            xall = xres.tile([128, 4, C], BF16)
            for g in range(NG):
                for c in range(4):
                    nc.sync.dma_start(
                        out=xall[:, c, g * 512 : (g + 1) * 512],
                        in_=xe[:, c, g * 512 : (g + 1) * 512],
                    )

            comb_all = xres.tile([128, 4 * NG], F32)

            # ---- gating prologue: comb for every chunk, one Exp table load
            for g in range(NG):
                lp = psT.tile([E, 512], F32, tag="tp")
                for c in range(4):
                    nc.tensor.matmul(
                        lp[:],
                        wg_sb[:, c, :],
                        xall[:, c, g * 512 : (g + 1) * 512],
                        start=(c == 0),
                        stop=(c == 3),
                    )
                l_sb = gate.tile([E, 512], F32, tag="lsb")
                nc.vector.tensor_scalar_add(l_sb[:], lp[:], bg_sb[:, 0:1])
                lt = gate.tile([128, 4, E], F32, tag="lt")
                for k in range(4):
                    tp = psT.tile([128, E], F32, tag="tp")
                    nc.tensor.transpose(
                        tp[:],
                        l_sb[:, k * 128 : (k + 1) * 128],
                        ident[:E, :E],
                    )
                    nc.vector.tensor_copy(out=lt[:, k, :], in_=tp[:])
                l0 = gate.tile([128, 4], F32, tag="l0")
                nc.vector.tensor_copy(out=l0[:], in_=lt[:, :, 0])
                d = gate.tile([128, 4, E], F32, tag="d")
                nc.vector.tensor_tensor(
                    out=d[:],
                    in0=lt[:],
                    in1=l0[:].to_broadcast([128, 4, E]),
                    op=mybir.AluOpType.subtract,
                )
                ex = gate.tile([128, 4, E], F32, tag="ex")
                nc.scalar.activation(ex[:], d[:], mybir.ActivationFunctionType.Exp)
                ssum = gate.tile([128, 4], F32, tag="ssum")
                nc.vector.tensor_reduce(
                    ssum[:], ex[:], axis=mybir.AxisListType.X, op=mybir.AluOpType.add
                )
                nc.vector.reciprocal(comb_all[:, 4 * g : 4 * g + 4], ssum[:])

            # ---- FFN phase (bf16), one Gelu table load, no input DMA
            for g in range(NG):
                hb = work.tile([128, NFT, 512], BF16, tag="hb")
                for ft in range(NFT):
                    hp = psA.tile([128, 512], F32, tag="mmA")
                    for hc in range(4):
                        nc.tensor.matmul(
                            hp[:],
                            w1_sb[:, ft, hc, :],
                            xall[:, hc, g * 512 : (g + 1) * 512],
                            start=(hc == 0),
                            stop=(hc == 3),
                        )
                    nc.scalar.activation(
                        hb[:, ft, :],
                        hp[:],
                        mybir.ActivationFunctionType.Gelu_apprx_tanh,
                        bias=b1_sb[:, ft : ft + 1],
                        scale=1.0,
                    )
                # second matmul emitted token-major: lhsT = h tile,
                # moving = W2 rows -> no output transposes needed
                for st in range(4):
                    yp = psB.tile([128, 512], F32, tag="mmB")
                    for fc in range(NFT):
                        nc.tensor.matmul(
                            yp[:],
                            hb[:, fc, st * 128 : (st + 1) * 128],
                            w2_sb[:, fc, :],
                            start=(fc == 0),
                            stop=(fc == NFT - 1),
                        )
                    y_sb = work.tile([128, H], F32, tag="ysb")
                    nc.vector.tensor_tensor(
                        out=y_sb[:], in0=yp[:], in1=b2_sb[:], op=mybir.AluOpType.add
                    )
                    nc.vector.tensor_scalar_mul(
                        y_sb[:], y_sb[:], comb_all[:, 4 * g + st : 4 * g + st + 1]
                    )
                    nc.sync.dma_start(
                        out=ypart[g * 512 + st * 128 : g * 512 + (st + 1) * 128, :],
                        in_=y_sb[:],
                    )
    nc.compile()
    return nc


def _route(xf, Wg, bg):
    """Top-2 routing on host (fp32, same semantics as the reference)."""
    logits = xf @ Wg + bg
    m = logits.max(-1, keepdims=True)
    p = np.exp(logits - m)
    p /= p.sum(-1, keepdims=True)
    order = np.argsort(-p, axis=-1, kind="stable")
    topi = order[:, :2]
    mask = np.zeros_like(p, dtype=bool)
    np.put_along_axis(mask, topi, True, axis=-1)
    idx = [np.nonzero(mask[:, e])[0] for e in range(E)]
    return idx


def _prep_inputs(x, Wg, bg, W1, b1, W2, b2):
    xf = np.ascontiguousarray(np.asarray(x, dtype=np.float32).reshape(T, H))
    Wg = np.asarray(Wg, dtype=np.float32)
    bg = np.asarray(bg, dtype=np.float32)
    W1 = np.asarray(W1, dtype=np.float32)
    b1 = np.asarray(b1, dtype=np.float32)
    W2 = np.asarray(W2, dtype=np.float32)
    b2 = np.asarray(b2, dtype=np.float32)

    idx = _route(xf, Wg, bg)
    maxc = max(len(i) for i in idx)
    C = max(512, -(-maxc // 512) * 512)

    # x transposed to [128, 4, T] (h = c*128 + p), bf16
    xbt = np.ascontiguousarray(
        np.transpose(xf.T.reshape(4, 128, T), (1, 0, 2))
    ).astype(ml_dtypes.bfloat16)

    in_maps = []
    for e in range(E):
        xg = np.zeros((128, 4, C), dtype=ml_dtypes.bfloat16)
        xg[:, :, : len(idx[e])] = xbt[:, :, idx[e]]
        perm = [e] + [j for j in range(E) if j != e]
        wg_p = Wg[:, perm]
        bg_p = bg[perm]
        # w1 F-tile-major: [128, ft, c, i] = W1[c*128+p, ft*128+i]
        w1r = np.ascontiguousarray(
            np.transpose(
                W1[e].reshape(4, 128, NFT, 128), (1, 2, 0, 3)
            ).astype(ml_dtypes.bfloat16)
        )
        in_maps.append(
            {
                "xe": xg,
                "wg": np.ascontiguousarray(
                    np.transpose(wg_p.reshape(4, 128, E), (1, 0, 2))
                ).astype(ml_dtypes.bfloat16),
                "bg": np.ascontiguousarray(bg_p.reshape(E, 1)),
                "w1": w1r,
                "b1t": np.ascontiguousarray(b1[e].reshape(NFT, 128).T),
                "w2": np.ascontiguousarray(
                    np.transpose(W2[e].reshape(NFT, 128, H), (1, 0, 2)).astype(
                        ml_dtypes.bfloat16
                    )
                ),
                "b2r": np.ascontiguousarray(
                    np.broadcast_to(b2[e][None, :], (128, H)).copy()
                ),
            }
        )
    return in_maps, idx, C


def kernel(x, Wg, bg, W1, b1, W2, b2):
    global LAST_RESULT
    in_maps, idx, C = _prep_inputs(x, Wg, bg, W1, b1, W2, b2)
    if C not in _CACHE:
        _CACHE[C] = _build(C)
    nc = _CACHE[C]
    import os

    trace = bool(os.environ.get("BASS_TRACE"))
    res = bass_utils.run_bass_kernel_spmd(
        nc, in_maps, core_ids=list(range(E)), trace=trace
    )
    LAST_RESULT = res
    out = np.zeros((T, H), dtype=np.float32)
    for e in range(E):
        out[idx[e]] += res.results[e]["ypart"][: len(idx[e])]
    return out.reshape(8, 2048, H)


# revision 8
# speedup vs baseline: 3.5400x; 1.0271x over previous
"""MoE (8 experts, top-2) TRN2 kernel — routed expert-parallel variant.

Sharding strategy (host = the shard/unshard glue): compute the top-2 routing
decision on host and shard tokens by expert id — core i receives exactly the
tokens routed to expert i (gathered, bf16, transposed), padded to a common
capacity C. Each core then computes, ON DEVICE, the gating softmax for its
tokens (to get the combine weight = raw softmax prob of its own expert), the
FFN in bf16, scales rows by the combine weight and writes y_part [C, H] in
bf16. Host scatter-adds the per-expert partials back to token order.

Gating prologue (high priority): per chunk, logits land in PSUM, get
transposed token-major and bias-added into one big tile; then a SINGLE Exp
activation covers all chunks (max 3 activation-table loads per run no matter
how the scheduler interleaves), comb = 1/sum_j exp(l_j - l_0) with gating
columns permuted per core so "my expert" is column 0.

DMA layout: x is chunk-major ([128, g, c, 512]) so each chunk is one big
contiguous DMA on the sync queue; weights ride the scalar queue in 4 large
DMAs; outputs ride the sync queue (idle during the FFN phase).
"""

import sys
import types

sys.path.insert(0, "/opt/trn_rl_repo")

import numpy as np
import ml_dtypes

try:
    import antenv.axon_hooks  # noqa: F401
except ImportError:
    try:
        import antenv
        import trn_agent_boot.trn_boot as _tb

        _hook = _tb._ntff_profile_via_ctypes("/opt/axon/libaxon_pjrt.so")
        _m = types.ModuleType("antenv.axon_hooks")
        _m.get_axon_ntff_profile_hook = lambda: _hook
        _m.set_axon_ntff_profile_hook = lambda h: None
        sys.modules["antenv.axon_hooks"] = _m
        antenv.axon_hooks = _m
    except Exception:
        pass

import concourse.bacc as bacc
import concourse.mybir as mybir
from concourse import bass, bass_utils
from concourse.tile import TileContext
from concourse.masks import make_identity

E = 8
H = 512
F = 2048
T = 8 * 2048
NFT = F // 128  # 16 F-tiles
BF16 = mybir.dt.bfloat16
F32 = mybir.dt.float32

_CACHE = {}
LAST_RESULT = None


def _build(C):
    """Bass program for one core: gating + FFN over C gathered tokens."""
    assert C % 512 == 0
    NG = C // 512
    nc = bacc.Bacc(debug=False)

    xe = nc.declare_dram_parameter("xe", [128, NG, 4, 512], BF16, isOutput=False)
    wg = nc.declare_dram_parameter("wg", [128, 4, E], BF16, isOutput=False)
    bg = nc.declare_dram_parameter("bg", [E, 1], F32, isOutput=False)
    # w1 F-tile-major: w1[p, ft, c, i] = W1[c*128+p, ft*128+i]
    w1 = nc.declare_dram_parameter("w1", [128, NFT, 4, 128], BF16, isOutput=False)
    b1t = nc.declare_dram_parameter("b1t", [128, NFT], F32, isOutput=False)
    w2 = nc.declare_dram_parameter("w2", [128, NFT, H], BF16, isOutput=False)
    b2r = nc.declare_dram_parameter("b2r", [128, H], F32, isOutput=False)
    ypart = nc.declare_dram_parameter("ypart", [C, H], BF16, isOutput=True)

    with TileContext(nc) as tc:
        with (
            tc.tile_pool(name="const", bufs=1) as constp,
            tc.tile_pool(name="xres", bufs=1) as xres,
            tc.tile_pool(name="work", bufs=4) as work,
            tc.tile_pool(name="gate", bufs=3) as gate,
            tc.tile_pool(name="psA", bufs=3, space="PSUM") as psA,
            tc.tile_pool(name="psB", bufs=3, space="PSUM") as psB,
            tc.tile_pool(name="psT", bufs=2, space="PSUM") as psT,
        ):
            ident = constp.tile([128, 128], F32)
            make_identity(nc, ident[:])
            wg_sb = constp.tile([128, 4, E], BF16)
            nc.sync.dma_start(out=wg_sb[:], in_=wg[:])
            bg_sb = constp.tile([E, 1], F32)
            nc.sync.dma_start(out=bg_sb[:], in_=bg[:])
            # weights ride the scalar (Activation) DMA queue in large pieces
            b1_sb = constp.tile([128, NFT], F32)
            nc.scalar.dma_start(out=b1_sb[:], in_=b1t[:])
            b2_sb = constp.tile([128, H], F32)
            nc.scalar.dma_start(out=b2_sb[:], in_=b2r[:])
            w1_sb = constp.tile([128, NFT, 4, 128], BF16)
            for hf in range(2):
                nc.scalar.dma_start(
                    out=w1_sb[:, hf * 8 : (hf + 1) * 8, :, :],
                    in_=w1[:, hf * 8 : (hf + 1) * 8, :, :],
                )
            w2_sb = constp.tile([128, NFT, H], BF16)
            for hf in range(2):
                nc.scalar.dma_start(
                    out=w2_sb[:, hf * 8 : (hf + 1) * 8, :],
                    in_=w2[:, hf * 8 : (hf + 1) * 8, :],
                )

            # all x chunks parked in SBUF: one large DMA per chunk
            xall = xres.tile([128, NG, 4, 512], BF16)
            for g in range(NG):
                nc.sync.dma_start(out=xall[:, g, :, :], in_=xe[:, g, :, :])

            lt_all = xres.tile([128, 4 * NG, E], F32)
            comb_all = xres.tile([128, 4 * NG], F32)

            # ---- gating prologue: logits+transpose per chunk, ONE Exp for all
            with tc.high_priority():
                for g in range(NG):
                    lp = psT.tile([E, 512], F32, tag="tp")
                    for c in range(4):
                        nc.tensor.matmul(
                            lp[:],
                            wg_sb[:, c, :],
                            xall[:, g, c, :],
                            start=(c == 0),
                            stop=(c == 3),
                        )
                    l_sb = gate.tile([E, 512], F32, tag="lsb")
                    nc.vector.tensor_scalar_add(l_sb[:], lp[:], bg_sb[:, 0:1])
                    for k in range(4):
                        tp = psT.tile([128, E], F32, tag="tp")
                        nc.tensor.transpose(
                            tp[:],
                            l_sb[:, k * 128 : (k + 1) * 128],
                            ident[:E, :E],
                        )
                        nc.vector.tensor_copy(out=lt_all[:, 4 * g + k, :], in_=tp[:])
                l0 = gate.tile([128, 4 * NG], F32, tag="l0")
                nc.vector.tensor_copy(out=l0[:], in_=lt_all[:, :, 0])
                d = gate.tile([128, 4 * NG, E], F32, tag="d")
                nc.vector.tensor_tensor(
                    out=d[:],
                    in0=lt_all[:],
                    in1=l0[:].to_broadcast([128, 4 * NG, E]),
                    op=mybir.AluOpType.subtract,
                )
                ex = gate.tile([128, 4 * NG, E], F32, tag="ex")
                nc.scalar.activation(ex[:], d[:], mybir.ActivationFunctionType.Exp)
                ssum = gate.tile([128, 4 * NG], F32, tag="ssum")
                nc.vector.tensor_reduce(
                    ssum[:], ex[:], axis=mybir.AxisListType.X, op=mybir.AluOpType.add
                )
                nc.vector.reciprocal(comb_all[:], ssum[:])

            # ---- FFN phase (bf16), one Gelu table load, no input DMA
            for g in range(NG):
                hb = work.tile([128, NFT, 512], BF16, tag="hb")
                for ft in range(NFT):
                    hp = psA.tile([128, 512], F32, tag="mmA")
                    for hc in range(4):
                        nc.tensor.matmul(
                            hp[:],
                            w1_sb[:, ft, hc, :],
                            xall[:, g, hc, :],
                            start=(hc == 0),
                            stop=(hc == 3),
                        )
                    nc.scalar.activation(
                        hb[:, ft, :],
                        hp[:],
                        mybir.ActivationFunctionType.Gelu_apprx_tanh,
                        bias=b1_sb[:, ft : ft + 1],
                        scale=1.0,
                    )
                # second matmul emitted token-major: lhsT = h tile,
                # moving = W2 rows -> no output transposes needed
                for st in range(4):
                    yp = psB.tile([128, 512], F32, tag="mmB")
                    for fc in range(NFT):
                        nc.tensor.matmul(
                            yp[:],
                            hb[:, fc, st * 128 : (st + 1) * 128],
                            w2_sb[:, fc, :],
                            start=(fc == 0),
                            stop=(fc == NFT - 1),
                        )
                    y32 = work.tile([128, H], F32, tag="y32")
                    nc.vector.tensor_tensor(
                        out=y32[:], in0=yp[:], in1=b2_sb[:], op=mybir.AluOpType.add
                    )
                    y_sb = work.tile([128, H], BF16, tag="ysb")
                    nc.vector.tensor_scalar_mul(
                        y_sb[:], y32[:], comb_all[:, 4 * g + st : 4 * g + st + 1]
                    )
                    nc.sync.dma_start(
                        out=ypart[g * 512 + st * 128 : g * 512 + (st + 1) * 128, :],
                        in_=y_sb[:],
                    )
    nc.compile()
    return nc


def _route(xf, Wg, bg):
    """Top-2 routing on host (fp32, same semantics as the reference)."""
    logits = xf @ Wg + bg
    m = logits.max(-1, keepdims=True)
    p = np.exp(logits - m)
    p /= p.sum(-1, keepdims=True)
    order = np.argsort(-p, axis=-1, kind="stable")
    topi = order[:, :2]
    mask = np.zeros_like(p, dtype=bool)
    np.put_along_axis(mask, topi, True, axis=-1)
    idx = [np.nonzero(mask[:, e])[0] for e in range(E)]
    return idx


def _prep_inputs(x, Wg, bg, W1, b1, W2, b2):
    xf = np.ascontiguousarray(np.asarray(x, dtype=np.float32).reshape(T, H))
    Wg = np.asarray(Wg, dtype=np.float32)
    bg = np.asarray(bg, dtype=np.float32)
    W1 = np.asarray(W1, dtype=np.float32)
    b1 = np.asarray(b1, dtype=np.float32)
    W2 = np.asarray(W2, dtype=np.float32)
    b2 = np.asarray(b2, dtype=np.float32)

    idx = _route(xf, Wg, bg)
    maxc = max(len(i) for i in idx)
    C = max(512, -(-maxc // 512) * 512)
    NG = C // 512

    # x transposed to [128, 4, T] (h = c*128 + p), bf16
    xbt = np.ascontiguousarray(
        np.transpose(xf.T.reshape(4, 128, T), (1, 0, 2))
    ).astype(ml_dtypes.bfloat16)

    in_maps = []
    for e in range(E):
        xg = np.zeros((128, 4, C), dtype=ml_dtypes.bfloat16)
        xg[:, :, : len(idx[e])] = xbt[:, :, idx[e]]
        # chunk-major: [128, g, c, 512]
        xg = np.ascontiguousarray(
            np.transpose(xg.reshape(128, 4, NG, 512), (0, 2, 1, 3))
        )
        perm = [e] + [j for j in range(E) if j != e]
        wg_p = Wg[:, perm]
        bg_p = bg[perm]
        # w1 F-tile-major: [128, ft, c, i] = W1[c*128+p, ft*128+i]
        w1r = np.ascontiguousarray(
            np.transpose(
                W1[e].reshape(4, 128, NFT, 128), (1, 2, 0, 3)
            ).astype(ml_dtypes.bfloat16)
        )
        in_maps.append(
            {
                "xe": xg,
                "wg": np.ascontiguousarray(
                    np.transpose(wg_p.reshape(4, 128, E), (1, 0, 2))
                ).astype(ml_dtypes.bfloat16),
                "bg": np.ascontiguousarray(bg_p.reshape(E, 1)),
                "w1": w1r,
                "b1t": np.ascontiguousarray(b1[e].reshape(NFT, 128).T),
                "w2": np.ascontiguousarray(
                    np.transpose(W2[e].reshape(NFT, 128, H), (1, 0, 2)).astype(
                        ml_dtypes.bfloat16
                    )
                ),
                "b2r": np.ascontiguousarray(
                    np.broadcast_to(b2[e][None, :], (128, H)).copy()
                ),
            }
        )
    return in_maps, idx, C


def kernel(x, Wg, bg, W1, b1, W2, b2):
    global LAST_RESULT
    in_maps, idx, C = _prep_inputs(x, Wg, bg, W1, b1, W2, b2)
    if C not in _CACHE:
        _CACHE[C] = _build(C)
    nc = _CACHE[C]
    import os

    trace = bool(os.environ.get("BASS_TRACE"))
    res = bass_utils.run_bass_kernel_spmd(
        nc, in_maps, core_ids=list(range(E)), trace=trace
    )
    LAST_RESULT = res
    out = np.zeros((T, H), dtype=np.float32)
    for e in range(E):
        out[idx[e]] += res.results[e]["ypart"][: len(idx[e])].astype(np.float32)
    return out.reshape(8, 2048, H)
